# revision 9
# baseline (speedup 1.0000x reference)
"""Deformable conv v1 Bass/Tile kernel for TRN2 (one sample per core).

V6 pipeline per core:
  prep:   cast input to bf16, HWDGE-transpose to channel-last directly into
          slot 0 of an SBUF quad-staging tile, build the other 3 corner slots
          via partition-shifted SBUF->SBUF copies, write one contiguous 8MB
          quad table to DRAM (partition-major entry order e' = p*32 + r).
          Table is borderless: boundary clamping is folded into swizzled
          per-slot bilinear weights.
  coords: P-layout slot weights (pixel%128 on partitions) with boundary
          swizzle; L2-layout -> PE fold/replicate -> int16 idx tiles
          (idx e' = (ey%2)*2048 + ex*32 + ey//2, ey/ex clamped to [0,62]).
  main:   8 chunks x 512 px; per tap: 1 dma_gather (512 idx x 2KB quad)
          -> 2 custom-DVE ops per 128-px block (u = TL*w0 + TR*w1,
             v = BL*w2 + BR*w3)
          -> PE transpose-accumulate via identity matmuls into PSUM
             (psum[c',pix] += u^T + v^T), ACT evacuates to colsT bf16
          -> 18-step matmul accumulation -> PSUM -> out
"""
import numpy as np
import ml_dtypes

import concourse.bass as bass
import concourse.tile as tile
from concourse import bacc, mybir

F32 = mybir.dt.float32
BF16 = mybir.dt.bfloat16
I16 = mybir.dt.int16

P = 128
H = W = 64
HW = H * W          # 4096
C = 256
O = 256
KK = 9              # 3x3 taps
SCH = 512           # pixels per chunk (gather + GEMM granularity)
NCH = HW // SCH     # 8 chunks
BIG = 12582912.0    # 1.5*2^23 round-trick constant (ulp=1 for |v| <= 2^22)

Alu = mybir.AluOpType
Act = mybir.ActivationFunctionType

USE_WSUM2 = True    # custom DVE op (Src0*C0 + Src1*C1); False -> ACT+stt path


# ---------------- custom DVE op ----------------

def _register_wsum2():
    """Register WSUM2_ANT: out = in0*s0 + in1*s1 (s0/s1 per-partition strips).

    Uses the documented extension point (dve_ops.OPS + name->row map); the
    uop program ships in the per-NEFF DVE table like every other custom op.
    """
    from concourse import dve_ops as D
    from concourse.dve_spec import Src0, Src1, C0, C1, Spec

    name = "WSUM2_ANT"
    for op in D.OPS:
        if op.name == name:
            return op
    spec = Spec(
        body=(Src0 * C0) + (Src1 * C1),
        reference=lambda in0, in1, s0, s1, imm2: (
            in0.astype(np.float32) * s0 + in1.astype(np.float32) * s1
        ),
    )
    op = D.DveOp(
        name, spec, False,
        {"v3": "f2ac165a27dbafb3", "v4": "49eb47656a95aba3"},
    )
    row = max(D._SUB_OPCODE_FOR_NAME.values()) + 1
    assert row < 0x20
    D.OPS.append(op)
    D._SUB_OPCODE_FOR_NAME[name] = row
    D.CUSTOM_DVE_SPECS[name] = spec
    return op


# ---------------- host constants ----------------

def _bases():
    """Base sampling grids. k = ky*3+kx; y_base = ky-1+row, x_base = kx-1+col."""
    n = np.arange(HW)
    ky = (np.arange(KK) // 3).astype(np.float32) - 1.0
    kx = (np.arange(KK) % 3).astype(np.float32) - 1.0
    yb = ky[None, :] + (n // W).astype(np.float32)[:, None]  # [HW, 9]
    xb = kx[None, :] + (n % W).astype(np.float32)[:, None]
    return yb, xb


def host_constants():
    yb, xb = _bases()
    # P-layout [128, 32, 9]: pixel n = 128*b + p
    ybp = yb.reshape(32, P, KK).transpose(1, 0, 2).copy()
    xbp = xb.reshape(32, P, KK).transpose(1, 0, 2).copy()
    # L2 layout [2, 128, 9, 16]: pixel n = 2048*cc + 16*Pp + f
    ybl = yb.reshape(2, P, 16, KK).transpose(0, 1, 3, 2).copy()
    xbl = xb.reshape(2, P, 16, KK).transpose(0, 1, 3, 2).copy()
    rep = (np.arange(P)[None, :] % 16 == np.arange(16)[:, None]).astype(np.float32)
    i128 = np.eye(P, dtype=np.float32)
    i128b = np.eye(P, dtype=np.float32).astype(ml_dtypes.bfloat16)
    # identity transpose-gather idx [128, 32] int16: id(q, s) = 16*s + q%16
    q = np.arange(P)[:, None]
    s = np.arange(SCH // 16)[None, :]
    idt = (16 * s + (q % 16)).astype(np.int16)
    return dict(ybp=ybp, xbp=xbp, ybl=ybl, xbl=xbl, rep=rep, i128=i128,
                i128b=i128b, idt=idt)


def host_weight(weight: np.ndarray) -> np.ndarray:
    """weight [O, C, 3, 3] f32 -> wt [128, 18, 256] bf16; B = k*2 + cb."""
    w = weight.reshape(O, C, KK)                        # k = ky*3+kx
    wt = np.empty((P, 2 * KK, O), dtype=np.float32)
    for k in range(KK):
        for cb in range(2):
            wt[:, k * 2 + cb, :] = w[:, cb * P:(cb + 1) * P, k].T
    return wt.astype(ml_dtypes.bfloat16)


# ---------------- build ----------------

def build(num_swdge_queues=4):
    if USE_WSUM2:
        _register_wsum2()
    nc = bacc.Bacc("TRN2", target_bir_lowering=False, debug=False,
                   num_devices=8, num_swdge_queues=num_swdge_queues)
    consts = host_constants()

    x = nc.dram_tensor("x", [C, HW], F32, kind="ExternalInput").ap()
    off = nc.dram_tensor("off", [18, HW], F32, kind="ExternalInput").ap()
    wt_d = nc.dram_tensor("wt", [P, 18, O], BF16, kind="ExternalInput").ap()
    y = nc.dram_tensor("y", [O, HW], F32, kind="ExternalOutput").ap()
    # quad table, partition-major entries: entry e' = p*32 + r lives at
    # qt[p, r, :, :]; content[j] = imgpix(n + {0,1,64,65}[j]) for n = 128r+p
    qt = nc.dram_tensor("qt", [P, 32, 4, C], BF16).ap()

    c_i128 = nc.inline_tensor(consts["i128"], "c_i128").ap()
    c_i128b = nc.inline_tensor(consts["i128b"], "c_i128b").ap()
    c_rep = nc.inline_tensor(consts["rep"], "c_rep").ap()
    c_ybp = nc.inline_tensor(consts["ybp"], "c_ybp").ap()
    c_xbp = nc.inline_tensor(consts["xbp"], "c_xbp").ap()
    c_ybl = nc.inline_tensor(consts["ybl"], "c_ybl").ap()
    c_xbl = nc.inline_tensor(consts["xbl"], "c_xbl").ap()

    with tile.TileContext(nc) as tc:
        _body(nc, tc, x, off, wt_d, y, qt,
              c_i128, c_i128b, c_rep, c_ybp, c_xbp, c_ybl, c_xbl)
    nc.compile()
    return nc


def _body(nc, tc, x, off, wt_d, y, qt,
          c_i128, c_i128b, c_rep, c_ybp, c_xbp, c_ybl, c_xbl):
    import contextlib
    wsum2 = None
    if USE_WSUM2:
        from concourse import dve_ops as D
        wsum2 = next(op for op in D.OPS if op.name == "WSUM2_ANT")

    ctx = contextlib.ExitStack()
    cpool = ctx.enter_context(tc.tile_pool(name="consts", bufs=1))
    i128_sb = cpool.tile([P, P], F32, tag="i128")
    nc.sync.dma_start(i128_sb[:], c_i128)
    i128b_sb = cpool.tile([P, P], BF16, tag="i128b")
    nc.sync.dma_start(i128b_sb[:], c_i128b)
    rep_sb = cpool.tile([16, P], F32, tag="rep")
    nc.sync.dma_start(rep_sb[:], c_rep)
    ybp_sb = cpool.tile([P, 32, KK], F32, tag="ybp")
    nc.sync.dma_start(ybp_sb[:], c_ybp)
    xbp_sb = cpool.tile([P, 32, KK], F32, tag="xbp")
    nc.sync.dma_start(xbp_sb[:], c_xbp)
    ybl_sb = cpool.tile([P, 2, KK, 16], F32, tag="ybl")
    nc.sync.dma_start(ybl_sb[:], c_ybl.transpose([1, 0, 2, 3]))
    xbl_sb = cpool.tile([P, 2, KK, 16], F32, tag="xbl")
    nc.sync.dma_start(xbl_sb[:], c_xbl.transpose([1, 0, 2, 3]))

    mpool = ctx.enter_context(tc.tile_pool(name="meta", bufs=1))
    off_l2v = off.rearrange("c (cc pp f) -> cc pp c f", pp=P, f=16)
    offL_t = mpool.tile([P, 2, 18, 16], F32, tag="offL")
    nc.sync.dma_start(offL_t[:, 0], off_l2v[0])
    nc.sync.dma_start(offL_t[:, 1], off_l2v[1])

    wt_sb = cpool.tile([P, 18, O], BF16, tag="wt")
    nc.scalar.dma_start(wt_sb[:], wt_d)

    pp_small_ctx = contextlib.ExitStack()
    pp_small = pp_small_ctx.enter_context(
        tc.tile_pool(name="ps_small", bufs=2, space="PSUM"))

    # ---------------- L2 layout -> gather idx tiles ----------------
    # idx value e' = (ey%2)*2048 + ex*32 + ey//2, ey/ex = clamp(floor, 0, 62)
    idx_sb = mpool.tile([P, 2, KK, P], I16, tag="idx")
    lpool_ctx = contextlib.ExitStack()
    lpool = lpool_ctx.enter_context(tc.tile_pool(name="l2", bufs=2))

    def emit_l2(cc):
        offL = offL_t[:, cc]

        def lfloor(v, nm):
            vr = lpool.tile([P, KK, 16], F32, tag=nm + "vr")
            nc.vector.tensor_scalar(vr[:], v[:], BIG, -BIG, Alu.add, Alu.add)
            d = lpool.tile([P, KK, 16], F32, tag=nm + "d")
            nc.vector.tensor_tensor(d[:], v[:], vr[:], Alu.subtract)
            ng = lpool.tile([P, KK, 16], F32, tag=nm + "ng")
            nc.vector.tensor_scalar(ng[:], d[:], 0.0, None, Alu.is_lt)
            v0 = lpool.tile([P, KK, 16], F32, tag=nm + "v0")
            nc.vector.tensor_tensor(v0[:], vr[:], ng[:], Alu.subtract)
            return v0

        yv = lpool.tile([P, KK, 16], F32, tag="lyv")
        nc.vector.tensor_tensor(yv[:], offL[:, 0:18:2, :], ybl_sb[:, cc],
                                Alu.add)
        y0 = lfloor(yv, "ly")
        xv = lpool.tile([P, KK, 16], F32, tag="lxv")
        nc.vector.tensor_tensor(xv[:], offL[:, 1:18:2, :], xbl_sb[:, cc],
                                Alu.add)
        x0 = lfloor(xv, "lx")
        ey = lpool.tile([P, KK, 16], F32, tag="ley")
        nc.vector.tensor_scalar(ey[:], y0[:], 0.0, 62.0, Alu.max, Alu.min)
        ex = lpool.tile([P, KK, 16], F32, tag="lex")
        nc.vector.tensor_scalar(ex[:], x0[:], 0.0, 62.0, Alu.max, Alu.min)
        # r = floor(ey/2) via round(ey*0.5 - 0.25); par = ey - 2r
        m = lpool.tile([P, KK, 16], F32, tag="lm")
        nc.vector.tensor_scalar(m[:], ey[:], 0.5, -0.25, Alu.mult, Alu.add)
        r = lpool.tile([P, KK, 16], F32, tag="lr")
        nc.vector.tensor_scalar(r[:], m[:], BIG, -BIG, Alu.add, Alu.add)
        r2 = lpool.tile([P, KK, 16], F32, tag="lr2")
        nc.vector.tensor_scalar(r2[:], r[:], 2.0, None, Alu.mult)
        par = lpool.tile([P, KK, 16], F32, tag="lpar")
        nc.vector.tensor_tensor(par[:], ey[:], r2[:], Alu.subtract)
        # e' = par*2048 + r + ex*32
        t1 = lpool.tile([P, KK, 16], F32, tag="lt1")
        nc.vector.scalar_tensor_tensor(t1[:], par[:], 2048.0, r[:],
                                       Alu.mult, Alu.add)
        eT = lpool.tile([P, KK, 16], F32, tag="leT")
        nc.vector.scalar_tensor_tensor(eT[:], ex[:], 32.0, t1[:],
                                       Alu.mult, Alu.add)
        for k in range(KK):
            psA = pp_small.tile([P, P], F32, tag="psA")
            nc.tensor.matmul(psA[0:16, :], eT[:, k, :], i128_sb[:],
                             start=True, stop=True)
            e16 = lpool.tile([16, P], F32, tag="e16")
            nc.vector.tensor_copy(e16[:], psA[0:16, :])
            psB = pp_small.tile([P, P], F32, tag="psB")
            nc.tensor.matmul(psB[:], rep_sb[:], e16[:],
                             start=True, stop=True)
            nc.vector.tensor_copy(idx_sb[:, cc, k, :], psB[:])

    emit_l2(0)
    emit_l2(1)
    lpool_ctx.close()

    # ---------------- P-layout slot weights with boundary swizzle ----------
    wmath_ctx = contextlib.ExitStack()
    wpool = wmath_ctx.enter_context(tc.tile_pool(name="wmath", bufs=1))

    offs = wpool.tile([18, HW], F32, tag="offs")
    nc.sync.dma_start(offs[:], off)
    offT = wpool.tile([P, 32, 18], F32, tag="offT")
    for b in range(32):
        pst = pp_small.tile([P, P], F32, tag="pssm")
        nc.tensor.matmul(pst[:, 0:18], offs[:, b * P:(b + 1) * P],
                         i128_sb[0:18, 0:18], start=True, stop=True)
        nc.vector.tensor_copy(offT[:, b, :], pst[:, 0:18])

    def floor_block(v, nm):
        vr = wpool.tile([P, 32, KK], F32, tag=nm + "vr")
        nc.vector.tensor_scalar(vr[:], v[:], BIG, -BIG, Alu.add, Alu.add)
        d = wpool.tile([P, 32, KK], F32, tag=nm + "d")
        nc.vector.tensor_tensor(d[:], v[:], vr[:], Alu.subtract)
        ng = wpool.tile([P, 32, KK], F32, tag=nm + "ng")
        nc.vector.tensor_scalar(ng[:], d[:], 0.0, None, Alu.is_lt)
        v0 = wpool.tile([P, 32, KK], F32, tag=nm + "v0")
        nc.vector.tensor_tensor(v0[:], vr[:], ng[:], Alu.subtract)
        fr = wpool.tile([P, 32, KK], F32, tag=nm + "fr")
        nc.vector.tensor_tensor(fr[:], v[:], v0[:], Alu.subtract)
        return v0, fr

    def axis_slot_weights(base_sb, chan0, nm):
        """per-slot weighted+masked pair (s0, s1) [128, 32, 9] for one axis.

        slot0 covers coord e = clamp(v0,0,62), slot1 covers e+1.
        wl = (1-fr)*[0<=v0<=63], wh = fr*[-1<=v0<=62];
        s0 = wl*[v0<=62] + wh*[v0==-1]; s1 = wh*[v0>=0] + wl*[v0==63].
        """
        v = wpool.tile([P, 32, KK], F32, tag=nm + "v")
        nc.vector.tensor_tensor(v[:], offT[:, :, chan0:18:2], base_sb[:],
                                Alu.add)
        v0, fr = floor_block(v, nm)
        wlo = wpool.tile([P, 32, KK], F32, tag=nm + "wlo")
        nc.vector.tensor_scalar(wlo[:], fr[:], -1.0, 1.0, Alu.mult, Alu.add)
        ge0 = wpool.tile([P, 32, KK], F32, tag=nm + "ge0")
        nc.vector.tensor_scalar(ge0[:], v0[:], -0.5, None, Alu.is_ge)
        le63 = wpool.tile([P, 32, KK], F32, tag=nm + "le63")
        nc.vector.tensor_scalar(le63[:], v0[:], 63.5, None, Alu.is_le)
        gem1 = wpool.tile([P, 32, KK], F32, tag=nm + "gem1")
        nc.vector.tensor_scalar(gem1[:], v0[:], -1.5, None, Alu.is_ge)
        le62 = wpool.tile([P, 32, KK], F32, tag=nm + "le62")
        nc.vector.tensor_scalar(le62[:], v0[:], 62.5, None, Alu.is_le)
        # wl = wlo * ge0 * le63 ; wh = fr * gem1 * le62
        t = wpool.tile([P, 32, KK], F32, tag=nm + "t")
        nc.vector.tensor_tensor(t[:], wlo[:], ge0[:], Alu.mult)
        wl = wpool.tile([P, 32, KK], F32, tag=nm + "wl")
        nc.vector.tensor_tensor(wl[:], t[:], le63[:], Alu.mult)
        t2 = wpool.tile([P, 32, KK], F32, tag=nm + "t2")
        nc.vector.tensor_tensor(t2[:], fr[:], gem1[:], Alu.mult)
        wh = wpool.tile([P, 32, KK], F32, tag=nm + "wh")
        nc.vector.tensor_tensor(wh[:], t2[:], le62[:], Alu.mult)
        # slot select
        lem1 = wpool.tile([P, 32, KK], F32, tag=nm + "lem1")
        nc.vector.tensor_scalar(lem1[:], v0[:], -0.5, None, Alu.is_le)
        ge63 = wpool.tile([P, 32, KK], F32, tag=nm + "ge63")
        nc.vector.tensor_scalar(ge63[:], v0[:], 62.5, None, Alu.is_ge)
        s0a = wpool.tile([P, 32, KK], F32, tag=nm + "s0a")
        nc.vector.tensor_tensor(s0a[:], wh[:], lem1[:], Alu.mult)
        s0b = wpool.tile([P, 32, KK], F32, tag=nm + "s0b")
        nc.vector.tensor_tensor(s0b[:], wl[:], le62[:], Alu.mult)
        s0 = wpool.tile([P, 32, KK], F32, tag=nm + "s0")
        nc.vector.tensor_tensor(s0[:], s0a[:], s0b[:], Alu.add)
        s1a = wpool.tile([P, 32, KK], F32, tag=nm + "s1a")
        nc.vector.tensor_tensor(s1a[:], wl[:], ge63[:], Alu.mult)
        s1b = wpool.tile([P, 32, KK], F32, tag=nm + "s1b")
        nc.vector.tensor_tensor(s1b[:], wh[:], ge0[:], Alu.mult)
        s1 = wpool.tile([P, 32, KK], F32, tag=nm + "s1")
        nc.vector.tensor_tensor(s1[:], s1a[:], s1b[:], Alu.add)
        return s0, s1

    sy0, sy1 = axis_slot_weights(ybp_sb, 0, "y")
    sx0, sx1 = axis_slot_weights(xbp_sb, 1, "x")
    # quad corner j: 0=(y,x) 1=(y,x+1) 2=(y+1,x) 3=(y+1,x+1)
    WTf = mpool.tile([P, 32, 4, KK], F32, tag="WTf")
    nc.vector.tensor_tensor(WTf[:, :, 0, :], sy0[:], sx0[:], Alu.mult)
    nc.vector.tensor_tensor(WTf[:, :, 1, :], sy0[:], sx1[:], Alu.mult)
    nc.vector.tensor_tensor(WTf[:, :, 2, :], sy1[:], sx0[:], Alu.mult)
    nc.vector.tensor_tensor(WTf[:, :, 3, :], sy1[:], sx1[:], Alu.mult)
    wmath_ctx.close()

    # ---------------- prep: quad staging in SBUF, table to DRAM ------------
    with tc.tile_pool(name="prep", bufs=1) as prep:
        # SWDGE cast-DMA: f32 DRAM -> bf16 SBUF directly (gpsimd is idle here)
        xb = prep.tile([P, 2, HW], BF16, tag="xb")
        for chh in range(2):
            for h2 in range(2):
                nc.gpsimd.dma_start(
                    xb[:, chh, h2 * 2048:(h2 + 1) * 2048],
                    x[chh * P:(chh + 1) * P, h2 * 2048:(h2 + 1) * 2048])
        # qtsb[p, r, j, :] = imgpix(128r + p + {0,1,64,65}[j])
        # slot 0 via PE transpose (identity matmul), evac on ACT/DVE
        qtsb = prep.tile([P, 32, 4, C], BF16, tag="qtsb")
        for b in range(32):
            pstp = pp_small.tile([P, 2, P], F32, tag="pstp")
            for chh in range(2):
                nc.tensor.matmul(pstp[:, chh, :],
                                 xb[:, chh, b * P:(b + 1) * P],
                                 i128b_sb[:], start=(chh == 0),
                                 stop=(chh == 1))
            for chh in range(2):
                dst = qtsb[:, b, 0, chh * P:(chh + 1) * P]
                if b % 2 == 0:
                    nc.scalar.copy(dst, pstp[:, chh, :])
                else:
                    nc.vector.tensor_copy(dst, pstp[:, chh, :])
        # corner shifts + table write, chasing the transposes in rank groups
        engs = [nc.sync, nc.scalar, nc.gpsimd]
        ei = 0

        def sdma(dst, srcv):
            nonlocal ei
            engs[ei % 3].dma_start(dst, srcv)
            ei += 1

        for rg in range(4):
            r0, r1 = rg * 8, rg * 8 + 8
            rr = slice(r0, r1)
            hi = min(r1 + 1, 32)
            # j=1: pixel n+1 -> partition p+1 (x=63 columns are don't-care)
            sdma(qtsb[0:127, rr, 1, :], qtsb[1:128, rr, 0, :])
            # j=2: pixel n+64: even y: p+64 same r; odd y: p-64, r+1
            sdma(qtsb[0:64, rr, 2, :], qtsb[64:128, rr, 0, :])
            sdma(qtsb[64:128, r0:hi - 1, 2, :],
                 qtsb[0:64, r0 + 1:hi, 0, :])
            # j=3: pixel n+65: even y: p+65; odd y: p-63, r+1
            sdma(qtsb[0:63, rr, 3, :], qtsb[65:128, rr, 0, :])
            sdma(qtsb[64:127, r0:hi - 1, 3, :],
                 qtsb[1:64, r0 + 1:hi, 0, :])
            eng = nc.sync if rg % 2 == 0 else nc.scalar
            eng.dma_start(qt[:, rr], qtsb[:, rr])

    # ---------------- main loop ----------------
    qt_ent = qt.rearrange("p r j c -> (p r) (j c)")   # entry e' = p*32 + r
    gpool = ctx.enter_context(tc.tile_pool(name="gather", bufs=6))
    upool = ctx.enter_context(tc.tile_pool(name="uv", bufs=12))
    ctpool = ctx.enter_context(tc.tile_pool(name="colsT", bufs=2))
    opool = ctx.enter_context(tc.tile_pool(name="outp", bufs=3))
    pp_small_ctx.close()
    pp_t = ctx.enter_context(tc.tile_pool(name="ps_t", bufs=5, space="PSUM"))
    pp_mm = ctx.enter_context(tc.tile_pool(name="ps_mm", bufs=2, space="PSUM"))

    y_v = y.rearrange("(oh p) (s n) -> oh p s n", oh=2, n=SCH)

    def emit_gemm(ch, colsT):
        for oh in range(2):
            ps = pp_mm.tile([P, SCH], F32, tag="psmm")
            for B in range(18):
                nc.tensor.matmul(ps[:], wt_sb[:, B, oh * P:(oh + 1) * P],
                                 colsT[:, B, :],
                                 start=(B == 0), stop=(B == 17))
            outt = opool.tile([P, SCH], F32, tag="outt")
            nc.scalar.copy(outt[:], ps[:])
            nc.sync.dma_start(y_v[oh, :, ch, :], outt[:])

    pending = None
    for ch in range(NCH):               # 8 chunks of 512 pixels
        cc = ch // 4
        q4 = ch % 4
        colsT = ctpool.tile([P, 18, SCH], BF16, tag="colsT")
        for k in range(KK):
            gt = gpool.tile([P, 4, 4 * C], BF16, tag="gt")
            nc.gpsimd.dma_gather(
                gt[:], qt_ent,
                idx_sb[:, cc, k, q4 * 32:(q4 + 1) * 32],
                num_idxs=SCH, num_idxs_reg=SCH,
                elem_size=4 * C, elem_step=4 * C,
                queue_num=(ch * KK + k) % 4)
            if k == 1 and pending is not None:
                emit_gemm(*pending)
                pending = None
            pst = [pp_t.tile([P, SCH], F32, name=f"pst{_i}", tag="pst")
                   for _i in range(2)]
            for blk in range(4):
                jblk = ch * 4 + blk
                uv = upool.tile([P, 2, C], BF16, tag="uv")
                if USE_WSUM2:
                    nc.vector._custom_dve(
                        wsum2, out=uv[:, 0, :],
                        in0=gt[:, blk, 0 * C:1 * C],
                        in1=gt[:, blk, 1 * C:2 * C],
                        s0=WTf[:, jblk, 0, k:k + 1],
                        s1=WTf[:, jblk, 1, k:k + 1])
                    nc.vector._custom_dve(
                        wsum2, out=uv[:, 1, :],
                        in0=gt[:, blk, 2 * C:3 * C],
                        in1=gt[:, blk, 3 * C:4 * C],
                        s0=WTf[:, jblk, 2, k:k + 1],
                        s1=WTf[:, jblk, 3, k:k + 1])
                else:
                    nc.scalar.activation(
                        uv[:, 0, :], gt[:, blk, 0 * C:1 * C], Act.Copy,
                        scale=WTf[:, jblk, 0, k:k + 1])
                    nc.vector.scalar_tensor_tensor(
                        uv[:, 0, :], gt[:, blk, 1 * C:2 * C],
                        WTf[:, jblk, 1, k:k + 1], uv[:, 0, :],
                        Alu.mult, Alu.add)
                    nc.scalar.activation(
                        uv[:, 1, :], gt[:, blk, 2 * C:3 * C], Act.Copy,
                        scale=WTf[:, jblk, 2, k:k + 1])
                    nc.vector.scalar_tensor_tensor(
                        uv[:, 1, :], gt[:, blk, 3 * C:4 * C],
                        WTf[:, jblk, 3, k:k + 1], uv[:, 1, :],
                        Alu.mult, Alu.add)
                # PE transpose-accumulate: psum[c', pix] += u^T + v^T
                for chh in range(2):
                    for s in range(2):
                        nc.tensor.matmul(
                            pst[chh][:, blk * P:(blk + 1) * P],
                            uv[:, s, chh * P:(chh + 1) * P],
                            i128b_sb[:],
                            start=(blk == 0 and s == 0),
                            stop=(blk == 3 and s == 1))
            for chh in range(2):
                nc.scalar.copy(colsT[:, k * 2 + chh, :], pst[chh][:])
        if ch == NCH - 1:
            if pending is not None:
                emit_gemm(*pending)
                pending = None
            emit_gemm(ch, colsT)
        else:
            pending = (ch, colsT)
    ctx.close()


# ---------------- harness entry point ----------------

_CACHED_NC = None


def _get_nc():
    global _CACHED_NC
    if _CACHED_NC is None:
        _CACHED_NC = build()
    return _CACHED_NC


def kernel(input, offset, weight):
    """Deformable conv v1 on 8 TRN2 cores, one sample per core.

    input  [8, 256, 64, 64] f32
    offset [8, 18, 64, 64]  f32
    weight [256, 256, 3, 3] f32
    -> [8, 256, 64, 64] f32
    """
    from concourse.bass_utils import run_bass_kernel_spmd
    input = np.ascontiguousarray(np.asarray(input, dtype=np.float32))
    offset = np.ascontiguousarray(np.asarray(offset, dtype=np.float32))
    weight = np.ascontiguousarray(np.asarray(weight, dtype=np.float32))
    nc = _get_nc()
    wt = host_weight(weight)
    in_maps = []
    for b in range(8):
        in_maps.append({
            "x": np.ascontiguousarray(input[b].reshape(C, HW)),
            "off": np.ascontiguousarray(offset[b].reshape(18, HW)),
            "wt": wt,
        })
    res = run_bass_kernel_spmd(nc, in_maps, core_ids=list(range(8)))
    out = np.stack([res.results[b]["y"].reshape(O, H, W) for b in range(8)])
    return out
